# revision 1
# baseline (speedup 1.0000x reference)
"""MLA absorbed-QKVO attention kernel for Trainium2 (8 NeuronCores).

Sharding: heads (H=16) tensor-parallel across 8 cores, 2 heads/core.
Host slices w_qb rows / w_o cols per core; each core computes a partial
output (its 2 heads through w_o) and the host sums the 8 partials.

v2: all transposes via PE (one-time cast+transpose pass writes transposed
bf16 weights/hidden to DRAM; per-block activation transposes via PE) —
no InstDmaTransposeAnt in steady state (HWDGE was the v1 bottleneck).
"""

import sys

import numpy as np

if "/opt/trn_rl_repo" not in sys.path:
    sys.path.insert(0, "/opt/trn_rl_repo")

B, S, HID = 2, 2048, 2048
H = 16
QK_ROPE = 64
KVR = 512
QLR = 1536
KVD = 640
DHEAD = 576
N_CORES = 8
HPC = H // N_CORES
OC = HPC * DHEAD    # 1152
SCALE = 1.0 / float(np.sqrt(128.0))

P = 128
SBLK = 512
KT = 512
NEG = -1e30


def build_nc(b_count=B, s_len=S):
    import concourse.bass as bass  # noqa: F401
    import concourse.mybir as mybir
    import concourse.tile as tile
    from concourse import bacc
    from concourse.masks import make_identity

    fp32 = mybir.dt.float32
    bf16 = mybir.dt.bfloat16
    Exp = mybir.ActivationFunctionType.Exp
    AX = mybir.AxisListType.X
    MAX = mybir.AluOpType.max
    GE = mybir.AluOpType.is_ge

    NB = s_len // SBLK
    NSC = SBLK // P
    NKC = HID // P
    NQLC = QLR // P
    NTOK = s_len // P
    R = b_count * s_len

    nc = bacc.Bacc(None, target_bir_lowering=False)

    hidden = nc.dram_tensor("hidden", [R, HID], fp32, kind="ExternalInput")
    ropeT_in = nc.dram_tensor("ropeT", [P, s_len], fp32, kind="ExternalInput")
    w_qa = nc.dram_tensor("w_qa", [QLR, HID], fp32, kind="ExternalInput")
    w_qb = nc.dram_tensor("w_qb_h", [OC, QLR], fp32, kind="ExternalInput")
    w_kv = nc.dram_tensor("w_kv", [KVD, HID], fp32, kind="ExternalInput")
    w_o = nc.dram_tensor("w_o_h", [HID, OC], fp32, kind="ExternalInput")
    out_d = nc.dram_tensor("out_part", [R, HID], fp32, kind="ExternalOutput")

    AV_CHUNKS = [
        [(0, 0, 64, 128), (1, 0, 192, 128), (2, 0, 320, 128),
         (3, 0, 448, 128), (4, 0, 576, 64)],
        [(4, 64, 64, 64), (5, 0, 128, 128), (6, 0, 256, 128),
         (7, 0, 384, 128), (8, 0, 512, 128)],
    ]
    QK_CHUNKS = [(0, 0, 64), (1, 64, 128), (2, 192, 128),
                 (3, 320, 128), (4, 448, 128)]

    with tile.TileContext(nc) as tc:
        with (
            tc.tile_pool(name="dram", bufs=1, space="DRAM") as dram,
            tc.tile_pool(name="singles", bufs=1) as singles,
            tc.tile_pool(name="cast", bufs=2) as cast,
            tc.tile_pool(name="strm", bufs=1) as strm,
            tc.tile_pool(name="work", bufs=1) as work,
            tc.tile_pool(name="stats", bufs=4) as stats,
            tc.tile_pool(name="rtmp", bufs=2) as rtmp,
            tc.tile_pool(name="psA", bufs=4, space="PSUM") as psA,
            tc.tile_pool(name="psS", bufs=1, space="PSUM") as psS,
            tc.tile_pool(name="psV", bufs=2, space="PSUM") as psV,
            tc.tile_pool(name="psT", bufs=1, space="PSUM") as psT,
        ):
            # transposed bf16 scratch in DRAM
            hbfT = [dram.tile([HID, SBLK], bf16, tag=f"hbfT{i}",
                              name=f"hbfT{i}") for i in range(R // SBLK)]
            w_qaT_d = dram.tile([HID, QLR], bf16, tag="wqaT", name="wqaT")
            w_qbT_d = dram.tile([QLR, OC], bf16, tag="wqbT", name="wqbT")
            w_kvT_d = dram.tile([HID, KVD], bf16, tag="wkvT", name="wkvT")
            w_oT_d = dram.tile([OC, HID], bf16, tag="woT", name="woT")

            ident = singles.tile([P, P], fp32, name="identf")
            make_identity(nc, ident[:, :])
            ident_b = singles.tile([P, P], bf16, name="identb")
            make_identity(nc, ident_b[:, :])

            cp_eng = [lambda o, i: nc.vector.tensor_copy(out=o, in_=i),
                      lambda o, i: nc.scalar.copy(out=o, in_=i)]
            cast_i = [0]

            def cast_T(dstT, src, rows, cols):
                """dstT[c, r] <- bf16(src[r, c]) via PE transpose.

                One load + one store per [128 x 1024] source tile; the store
                covers 8 transposed row-chunks of dstT via a strided AP."""
                for r0 in range(0, rows, P):
                    for c0 in range(0, cols, 1024):
                        cw = min(1024, cols - c0)
                        ti = work.tile([P, 1024], fp32, tag="srow", bufs=2,
                                       name="ci")
                        nc.sync.dma_start(
                            out=ti[:, :cw], in_=src[r0:r0 + P, c0:c0 + cw])
                        so = cast.tile([P, 1024], bf16, tag="cso", name="cso")
                        for g in range(0, cw, 512):
                            gw = min(512, cw - g)
                            ps = psS.tile([P, KT], fp32, tag="psS", name="psS")
                            for j in range(gw // P):
                                nc.tensor.transpose(
                                    ps[:, j * P:(j + 1) * P],
                                    ti[:, g + j * P:g + (j + 1) * P],
                                    ident[:, :])
                            eng = cp_eng[cast_i[0] % 2]
                            cast_i[0] += 1
                            eng(so[:, g:g + gw], ps[:, :gw])
                        nc.gpsimd.dma_start(
                            out=dstT[c0:c0 + cw, r0:r0 + P].rearrange(
                                "(a p) r -> p a r", p=P),
                            in_=so[:, :cw].rearrange("p (a r) -> p a r", r=P))

            cast_T(w_qaT_d, w_qa, QLR, HID)
            cast_T(w_kvT_d, w_kv, KVD, HID)
            cast_T(w_qbT_d, w_qb, OC, QLR)
            cast_T(w_oT_d, w_o, HID, OC)
            for i in range(R // SBLK):
                cast_T(hbfT[i], hidden[i * SBLK:(i + 1) * SBLK, :], SBLK, HID)

            ropeT = singles.tile([P, s_len], fp32, name="ropeTs")
            nc.sync.dma_start(out=ropeT[:, :], in_=ropeT_in[:, :])

            def rope_apply(dst_hi, dst_lo, src, cols):
                w = src.shape[-1]
                a = rtmp.tile([32, P], fp32, tag="rt0", name="rt0")
                bb = rtmp.tile([32, P], fp32, tag="rt1", name="rt1")
                nc.vector.tensor_mul(a[:, :w], src[0:32, :], ropeT[0:32, cols])
                nc.vector.tensor_mul(bb[:, :w], src[32:64, :],
                                     ropeT[64:96, cols])
                nc.vector.tensor_sub(dst_hi, a[:, :w], bb[:, :w])
                nc.vector.tensor_mul(a[:, :w], src[32:64, :],
                                     ropeT[32:64, cols])
                nc.vector.tensor_mul(bb[:, :w], src[0:32, :],
                                     ropeT[96:128, cols])
                nc.vector.tensor_add(dst_lo, a[:, :w], bb[:, :w])

            # ---------------- main loop ----------------
            for b in range(b_count):
                kv_r = work.tile([P, NTOK, KVD], bf16, tag="kv_r",
                                 name="kv_r")
                keyT = work.tile([P, 5, s_len], bf16, tag="keyT", name="keyT")

                for blk in range(NB):
                    rows0 = b * s_len + blk * SBLK
                    nblk = b * NB + blk

                    hidT = work.tile([P, NKC, SBLK], bf16, tag="hidT",
                                     bufs=2, name="hidT")
                    nc.sync.dma_start(
                        out=hidT[:, :, :],
                        in_=hbfT[nblk].rearrange("(a p) s -> p a s", p=P))

                    # ---- q_a = hidden @ w_qa.T (row-major) ----
                    q_a_sb = work.tile([P, NSC, QLR], bf16, tag="q_a_sb",
                                       name="q_a_sb")
                    for ct in range(QLR // KT):
                        pss = [psA.tile([P, KT], fp32, tag="psA", name="psA")
                               for _ in range(NSC)]
                        for kc2 in range(NKC // 2):
                            wt = strm.tile([P, 2, KT], bf16, tag="w_qaT",
                                           bufs=4, name="w_qaT")
                            nc.sync.dma_start(
                                out=wt[:, :, :],
                                in_=w_qaT_d[kc2 * 2 * P:(kc2 + 1) * 2 * P,
                                            ct * KT:(ct + 1) * KT].rearrange(
                                                "(a p) q -> p a q", p=P))
                            for i in range(2):
                                kc = kc2 * 2 + i
                                for sc in range(NSC):
                                    nc.tensor.matmul(
                                        pss[sc][:, :],
                                        hidT[:, kc, sc * P:(sc + 1) * P],
                                        wt[:, i, :],
                                        start=(kc == 0), stop=(kc == NKC - 1))
                        for sc in range(NSC):
                            nc.scalar.copy(
                                out=q_a_sb[:, sc, ct * KT:(ct + 1) * KT],
                                in_=pss[sc][:, :])

                    # ---- kv = hidden @ w_kv.T (row-major) ----
                    for noff, nw in [(0, 512), (512, 128)]:
                        pss = [psA.tile([P, KT], fp32, tag="psA", name="psA")
                               for _ in range(NSC)]
                        for kc in range(NKC):
                            wt = strm.tile([P, KVD], bf16, tag="w_kvT",
                                           bufs=3, name="w_kvT")
                            nc.sync.dma_start(
                                out=wt[:, :],
                                in_=w_kvT_d[kc * P:(kc + 1) * P, :])
                            for sc in range(NSC):
                                nc.tensor.matmul(
                                    pss[sc][:, :nw],
                                    hidT[:, kc, sc * P:(sc + 1) * P],
                                    wt[:, noff:noff + nw],
                                    start=(kc == 0), stop=(kc == NKC - 1))
                        for sc in range(NSC):
                            nc.vector.tensor_copy(
                                out=kv_r[:, blk * NSC + sc, noff:noff + nw],
                                in_=pss[sc][:, :nw])

                    # ---- keyT assembly: nope via PE transpose (dedicated
                    # bf16 PSUM pool; no f32/bf16 slot sharing) ----
                    bcols = slice(blk * SBLK, (blk + 1) * SBLK)
                    for j in range(4):
                        ps = psT.tile([P, KT], bf16, tag="psT", name="psT")
                        for sc in range(NSC):
                            nc.tensor.transpose(
                                ps[:, sc * P:(sc + 1) * P],
                                kv_r[:, blk * NSC + sc,
                                     128 + j * P:128 + (j + 1) * P],
                                ident_b[:, :])
                        nc.scalar.copy(out=keyT[:, 1 + j, bcols], in_=ps[:, :])
                    for sc in range(NSC):
                        tkc = blk * NSC + sc
                        cols = slice(blk * SBLK + sc * P,
                                     blk * SBLK + (sc + 1) * P)
                        t64 = rtmp.tile([P, 64], fp32, tag="t64", name="t64")
                        nc.vector.tensor_copy(out=t64[:, :],
                                              in_=kv_r[:, tkc, 0:64])
                        pt = psS.tile([P, KT], fp32, tag="psS", name="psS")
                        nc.tensor.transpose(pt[0:64, 0:P], t64[:, :],
                                            ident[:, :])
                        rope_apply(keyT[0:32, 0, cols], keyT[32:64, 0, cols],
                                   pt[0:64, 0:P], cols)

                    # ---- q_aT via PE transpose ----
                    q_aT = work.tile([P, NQLC, SBLK], bf16, tag="q_aT",
                                     name="q_aT")
                    for qlc in range(NQLC):
                        ps = psT.tile([P, KT], bf16, tag="psT", name="psT")
                        for sc in range(NSC):
                            nc.tensor.transpose(
                                ps[:, sc * P:(sc + 1) * P],
                                q_a_sb[:, sc, qlc * P:(qlc + 1) * P],
                                ident_b[:, :])
                        nc.vector.tensor_copy(out=q_aT[:, qlc, :],
                                              in_=ps[:, :])

                    # ---- q = q_a @ w_qb.T (row-major) ----
                    q_sb = work.tile([P, NSC, OC], bf16, tag="q_sb",
                                     name="q_sb")
                    for ooff, otw in [(0, 512), (512, 512), (1024, 128)]:
                        pss = [psA.tile([P, KT], fp32, tag="psA", name="psA")
                               for _ in range(NSC)]
                        for qlc2 in range(NQLC // 2):
                            wt = strm.tile([P, 2, KT], bf16, tag="w_qbT",
                                           bufs=4, name="w_qbT")
                            nc.sync.dma_start(
                                out=wt[:, :, :otw],
                                in_=w_qbT_d[qlc2 * 2 * P:(qlc2 + 1) * 2 * P,
                                            ooff:ooff + otw].rearrange(
                                                "(a p) q -> p a q", p=P))
                            for i in range(2):
                                qlc = qlc2 * 2 + i
                                for sc in range(NSC):
                                    nc.tensor.matmul(
                                        pss[sc][:, :otw],
                                        q_aT[:, qlc, sc * P:(sc + 1) * P],
                                        wt[:, i, :otw],
                                        start=(qlc == 0),
                                        stop=(qlc == NQLC - 1))
                        for sc in range(NSC):
                            nc.scalar.copy(
                                out=q_sb[:, sc, ooff:ooff + otw],
                                in_=pss[sc][:, :otw])

                    # ---- queryT (d-major per head) + RoPE ----
                    queryT = work.tile([P, 2 * 5, SBLK], bf16, tag="queryT",
                                       name="queryT")
                    for hh in range(HPC):
                        for slot, doff, dw in QK_CHUNKS[1:]:
                            so = hh * DHEAD + doff
                            ps = psT.tile([P, KT], bf16, tag="psT", name="psT")
                            for sc in range(NSC):
                                nc.tensor.transpose(
                                    ps[:, sc * P:(sc + 1) * P],
                                    q_sb[:, sc, so:so + dw], ident_b[:, :])
                            nc.scalar.copy(out=queryT[:, hh * 5 + slot, :],
                                           in_=ps[:, :])
                        for sc in range(NSC):
                            lcols = slice(sc * P, (sc + 1) * P)
                            gcols = slice(blk * SBLK + sc * P,
                                          blk * SBLK + (sc + 1) * P)
                            t64 = rtmp.tile([P, 64], fp32, tag="t64",
                                            name="t64")
                            nc.vector.tensor_copy(
                                out=t64[:, :],
                                in_=q_sb[:, sc, hh * DHEAD:hh * DHEAD + 64])
                            pt = psS.tile([P, KT], fp32, tag="psS", name="psS")
                            nc.tensor.transpose(pt[0:64, 0:P], t64[:, :],
                                                ident[:, :])
                            rope_apply(queryT[0:32, hh * 5, lcols],
                                       queryT[32:64, hh * 5, lcols],
                                       pt[0:64, 0:P], gcols)

                    # ---- attention per head ----
                    attnoutT = work.tile([P, 9, SBLK], bf16, tag="attnoutT",
                                         name="attnoutT")
                    for hh in range(HPC):
                        PT = work.tile([P, NTOK, SBLK], bf16, tag="PT",
                                       name="PT")
                        for sc in range(NSC):
                            klen = (blk + 1) * KT
                            srow = work.tile([P, s_len], fp32, tag="srow",
                                             bufs=2, name="srow")
                            for kt in range(blk + 1):
                                ps_s = psV.tile([P, KT], fp32,
                                                tag="psV", name="ps_s")
                                for slot, doff, dw in QK_CHUNKS:
                                    nc.tensor.matmul(
                                        ps_s[:, :],
                                        queryT[0:dw, hh * 5 + slot,
                                               sc * P:(sc + 1) * P],
                                        keyT[0:dw, slot,
                                             kt * KT:(kt + 1) * KT],
                                        start=(slot == 0), stop=(slot == 4))
                                nc.vector.tensor_copy(
                                    out=srow[:, kt * KT:(kt + 1) * KT],
                                    in_=ps_s[:, :])
                            nc.gpsimd.affine_select(
                                out=srow[:, blk * KT:(blk + 1) * KT],
                                in_=srow[:, blk * KT:(blk + 1) * KT],
                                compare_op=GE, fill=NEG, base=sc * P,
                                pattern=[[-1, KT]], channel_multiplier=1)
                            mx = stats.tile([P, 1], fp32, tag="mx", name="mx")
                            nc.vector.tensor_reduce(
                                mx[:, :], srow[:, 0:klen], axis=AX, op=MAX)
                            negb = stats.tile([P, 1], fp32, tag="negb",
                                              name="negb")
                            nc.vector.tensor_scalar_mul(
                                negb[:, :], mx[:, :], -SCALE)
                            ssum = stats.tile([P, 1], fp32, tag="ssum",
                                              name="ssum")
                            nc.scalar.activation(
                                srow[:, 0:klen], srow[:, 0:klen], Exp,
                                bias=negb[:, :], scale=SCALE,
                                accum_out=ssum[:, :])
                            rec = stats.tile([P, 1], fp32, tag="rec",
                                             name="rec")
                            nc.vector.reciprocal(rec[:, :], ssum[:, :])
                            nc.vector.tensor_scalar_mul(
                                srow[:, 0:klen], srow[:, 0:klen], rec[:, :])
                            for g in range(blk + 1):
                                ps_t = psS.tile([P, KT], fp32, tag="psS",
                                                name="psS")
                                for j in range(4):
                                    nc.tensor.transpose(
                                        ps_t[:, j * P:(j + 1) * P],
                                        srow[:, (g * 4 + j) * P:
                                             (g * 4 + j + 1) * P],
                                        ident[:, :])
                                nc.vector.tensor_copy(
                                    out=PT[:, g * 4:(g + 1) * 4,
                                           sc * P:(sc + 1) * P],
                                    in_=ps_t.rearrange("p (a c) -> p a c",
                                                       a=4))
                        ntok_b = (blk + 1) * NSC
                        for ci, ro, vo, vw in AV_CHUNKS[hh]:
                            ps_v = psV.tile([P, SBLK], fp32, tag="psV",
                                            name="psV")
                            for tkc in range(ntok_b):
                                nc.tensor.matmul(
                                    ps_v[0:vw, :],
                                    kv_r[:, tkc, vo:vo + vw],
                                    PT[:, tkc, :],
                                    start=(tkc == 0), stop=(tkc == ntok_b - 1))
                            nc.scalar.copy(
                                out=attnoutT[ro:ro + vw, ci, :],
                                in_=ps_v[0:vw, :])

                    # ---- out_partial = attnout @ w_o.T ----
                    for ct in range(HID // KT):
                        wots = []
                        for hd2 in range(5):
                            n = 2 if hd2 < 4 else 1
                            wt = strm.tile([P, 2, KT], bf16, tag="w_oT",
                                           bufs=5, name="w_oT")
                            nc.sync.dma_start(
                                out=wt[:, :n, :],
                                in_=w_oT_d[hd2 * 2 * P:hd2 * 2 * P + n * P,
                                           ct * KT:(ct + 1) * KT].rearrange(
                                               "(a p) q -> p a q", p=P))
                            for i in range(n):
                                wots.append(wt[:, i, :])
                        for sc in range(NSC):
                            ps_o = psV.tile([P, SBLK], fp32, tag="psV",
                                            name="ps_o")
                            for hd in range(9):
                                nc.tensor.matmul(
                                    ps_o[:, :],
                                    attnoutT[:, hd, sc * P:(sc + 1) * P],
                                    wots[hd][:, :],
                                    start=(hd == 0), stop=(hd == 8))
                            ot = work.tile([P, KT], fp32, tag="out_sb",
                                           bufs=2, name="out_sb")
                            nc.vector.tensor_copy(out=ot[:, :], in_=ps_o[:, :])
                            nc.gpsimd.dma_start(
                                out=out_d[rows0 + sc * P:rows0 + (sc + 1) * P,
                                          ct * KT:(ct + 1) * KT],
                                in_=ot[:, :])

    nc.compile()
    return nc


def make_in_maps(inputs, b_count=B, s_len=S):
    hidden = np.ascontiguousarray(
        np.asarray(inputs["hidden_states"], dtype=np.float32).reshape(
            b_count * s_len, HID))
    cos = np.asarray(inputs["cos"], dtype=np.float32)
    sin = np.asarray(inputs["sin"], dtype=np.float32)
    ropeT = np.ascontiguousarray(
        np.concatenate([cos[0].T, sin[0].T], axis=0))  # [128, s_len]
    w_qa = np.ascontiguousarray(np.asarray(inputs["w_qa"], np.float32))
    w_qb = np.asarray(inputs["w_qb"], dtype=np.float32)
    w_kv = np.ascontiguousarray(np.asarray(inputs["w_kv"], np.float32))
    w_o = np.asarray(inputs["w_o"], dtype=np.float32)
    in_maps = []
    for c in range(N_CORES):
        in_maps.append({
            "hidden": hidden,
            "ropeT": ropeT,
            "w_qa": w_qa,
            "w_qb_h": np.ascontiguousarray(w_qb[c * OC:(c + 1) * OC, :]),
            "w_kv": w_kv,
            "w_o_h": np.ascontiguousarray(w_o[:, c * OC:(c + 1) * OC]),
        })
    return in_maps


_NC_CACHE = {}


def run_on_hw(inputs, trace=False):
    import os

    from concourse.bass_utils import run_bass_kernel_spmd

    if not trace:
        # axon client has no NTFF hook; a stray BASS_TRACE=1 would crash.
        os.environ["BASS_NEVER_TRACE"] = "1"

    key = "full"
    if key not in _NC_CACHE:
        _NC_CACHE[key] = build_nc()
    nc = _NC_CACHE[key]
    in_maps = make_in_maps(inputs)
    res = run_bass_kernel_spmd(nc, in_maps, core_ids=list(range(N_CORES)),
                               trace=trace)
    acc = np.zeros((B * S, HID), dtype=np.float32)
    for r in res.results:
        acc += r["out_part"]
    return acc.reshape(B, S, HID), res


def kernel(**inputs):
    out, _ = run_on_hw(inputs, trace=False)
    return out



# revision 33
# speedup vs baseline: 2.0536x; 2.0536x over previous
"""MLA absorbed-QKVO attention kernel for Trainium2 (8 NeuronCores), v3.

Sharding: heads (H=16) tensor-parallel across 8 cores, 2 heads/core.
Host fuses W_h = w_qb_h @ w_qa (per-core), pre-transposes/casts all
weights + hidden to bf16, and builds swizzled rope tables + causal
masks. Each core computes a partial output (its 2 heads through w_o);
the host sums the 8 partials.

Device dataflow is weights-stationary / d-major throughout:
  q^T       = W_hT  x hidT      (PSUM -> queryT slots, rope on d-rows)
  kv^T      = w_kvT x hidT      (PSUM -> keyT slots + V via PE transpose)
  scores^T  = keyT^T x queryT   (PSUM -> exp -> P^T bf16, no-max softmax)
  attnout   = P^T^T x V         (q-major PSUM; 1/sum via per-partition
                                 scale at evacuation; ones-column of V
                                 gives the softmax denominator for free)
  out       = attnoutT^T x w_oT (after a small PE transpose of attnout)
"""

import sys

import numpy as np

if "/opt/trn_rl_repo" not in sys.path:
    sys.path.insert(0, "/opt/trn_rl_repo")

import ml_dtypes

BF = ml_dtypes.bfloat16

B, S, HID = 2, 2048, 2048
H = 16
QK_ROPE = 64
KVR = 512
QLR = 1536
KVD = 640
DHEAD = 576
N_CORES = 8
HPC = H // N_CORES
OC = HPC * DHEAD      # 1152
OCP = HPC * 640       # 1280 (per-head padded to 5x128)
SCALE = 1.0 / float(np.sqrt(128.0))

P = 128
SBLK = 512


def build_nc(b_count=B, s_len=S, debug=False, stage=3):
    import concourse.bass as bass  # noqa: F401
    import concourse.mybir as mybir
    import concourse.tile as tile
    from concourse import bacc
    from concourse.masks import make_identity

    fp32 = mybir.dt.float32
    bf16 = mybir.dt.bfloat16
    Exp = mybir.ActivationFunctionType.Exp
    Copy = mybir.ActivationFunctionType.Copy

    NB = s_len // SBLK          # blocks per batch
    NKC = HID // P              # 16 hid chunks
    NTOKB = s_len // P          # k sub-chunks per batch
    R = b_count * s_len

    nc = bacc.Bacc(None, target_bir_lowering=False)

    hidT_d = nc.dram_tensor("hidT", [HID, R], bf16, kind="ExternalInput")
    whT_d = nc.dram_tensor("whT", [HID, OC], bf16, kind="ExternalInput")
    wkvT_d = nc.dram_tensor("wkvT", [HID, KVD], bf16, kind="ExternalInput")
    woT_d = nc.dram_tensor("woT", [OCP, HID], bf16, kind="ExternalInput")
    ropeT_d = nc.dram_tensor("ropeT", [P, s_len], fp32, kind="ExternalInput")
    maskT_d = nc.dram_tensor("maskT", [P, 4, SBLK], bf16,
                             kind="ExternalInput")
    out_d = nc.dram_tensor("out_part", [R, HID], fp32, kind="ExternalOutput")
    if debug:
        NTOKB_ = s_len // P
        dbg_v = nc.dram_tensor("dbg_v", [P, NTOKB_, 577], bf16,
                               kind="ExternalOutput")
        dbg_key = nc.dram_tensor("dbg_key", [P, 5, s_len], bf16,
                                 kind="ExternalOutput")
        dbg_q = nc.dram_tensor("dbg_q", [P, 10, SBLK], bf16,
                               kind="ExternalOutput")
        dbg_ept = nc.dram_tensor("dbg_ept", [P, SBLK], bf16,
                                 kind="ExternalOutput")
        dbg_avt = nc.dram_tensor("dbg_avt", [P, 4, 65], fp32,
                                 kind="ExternalOutput")

    with tile.TileContext(nc) as tc:
        with (
            tc.tile_pool(name="singles", bufs=1) as singles,
            tc.tile_pool(name="batch", bufs=1) as batch,
            tc.tile_pool(name="work", bufs=1) as work,
            tc.tile_pool(name="strm", bufs=1) as strm,
            tc.tile_pool(name="stats", bufs=8) as stats,
            tc.tile_pool(name="psQ", bufs=2, space="PSUM") as psQ,
            tc.tile_pool(name="psAV", bufs=4, space="PSUM") as psAV,
            tc.tile_pool(name="psT", bufs=2, space="PSUM") as psT,
        ):
            # ---- resident weights / tables ----
            whT = singles.tile([P, NKC, OC], bf16, name="whT")
            nc.sync.dma_start(
                out=whT[:, :, :],
                in_=whT_d.rearrange("(a p) m -> p a m", p=P))
            wkvT = singles.tile([P, NKC, KVD], bf16, name="wkvT")
            nc.sync.dma_start(
                out=wkvT[:, :, :],
                in_=wkvT_d.rearrange("(a p) m -> p a m", p=P))
            woT = singles.tile([P, 10, HID], bf16, name="woT")
            nc.sync.dma_start(
                out=woT[:, :, :],
                in_=woT_d.rearrange("(a p) m -> p a m", p=P))
            ropeT = singles.tile([P, s_len], fp32, name="ropeT")
            nc.sync.dma_start(out=ropeT[:, :], in_=ropeT_d[:, :])
            maskT = singles.tile([P, 4, SBLK], bf16, name="maskT")
            nc.sync.dma_start(out=maskT[:, :, :], in_=maskT_d[:, :, :])
            identb = singles.tile([P, P], bf16, name="identb")
            make_identity(nc, identb[:, :])

            def rope_apply(dst_hi, dst_lo, src0, src32, cols):
                """dst rows <- rope(src [64 PSUM rows; src0=rows 0:32,
                src32=rows 32:64 at any partition base]).

                Table: rows 0:64 cos, 64:128 swizzled sin (see make_in_maps).
                m2 is written half-swapped so every SBUF+SBUF op below has
                equal input base partitions (a walrus verifier requirement).
                """
                m1 = strm.tile([64, SBLK], bf16, tag="m1", bufs=2, name="m1")
                m2 = strm.tile([64, SBLK], bf16, tag="m2", bufs=2, name="m2")
                nc.vector.tensor_mul(m1[0:32, :], src0, ropeT[0:32, cols])
                nc.vector.tensor_mul(m1[32:64, :], src32, ropeT[32:64, cols])
                nc.vector.tensor_mul(m2[32:64, :], src0, ropeT[64:96, cols])
                nc.vector.tensor_mul(m2[0:32, :], src32, ropeT[96:128, cols])
                nc.vector.tensor_sub(dst_hi, m1[0:32, :], m2[0:32, :])
                nc.vector.tensor_add(dst_lo, m1[32:64, :], m2[32:64, :])

            for b in range(b_count):
                keyT = batch.tile([P, 5, s_len], bf16, tag="keyT",
                                  name="keyT")
                # rows 64:128 of slot4 are never real data, but the V
                # transpose reads the full 128 rows (K=64 PE transposes
                # fail on hw); keep them finite.
                nc.gpsimd.memset(keyT[64:128, 4, :], 0.0)
                V = batch.tile([P, NTOKB, 577], bf16, tag="V", name="V")
                nc.gpsimd.memset(V[:, :, 576:577], 1.0)

                for blk in range(NB):
                    tok0 = blk * SBLK
                    rows0 = b * s_len + tok0
                    bcols = slice(tok0, tok0 + SBLK)

                    hidT = work.tile([P, NKC, SBLK], bf16, tag="hidT",
                                     bufs=1, name="hidT")
                    nc.sync.dma_start(
                        out=hidT[:, :, :],
                        in_=hidT_d[:, rows0:rows0 + SBLK].rearrange(
                            "(a p) s -> p a s", p=P))

                    # ---- kv projection -> keyT slots (+rope) + vk0 ----
                    # vk0 holds V dims 0:128 d-major (= [v_rope; nope 0:64])
                    # so every V transpose below is a full-K=128 transpose
                    # (K=64 PE transposes fail at runtime on this backend).
                    vk0 = work.tile([P, SBLK], bf16, tag="vk0",
                                    bufs=2, name="vk0")
                    for c in range(5):
                        ps = psQ.tile([P, SBLK], fp32, tag="psQ", name="psQ")
                        for a in range(NKC):
                            nc.tensor.matmul(
                                ps[:, :], wkvT[:, a, c * P:(c + 1) * P],
                                hidT[:, a, :],
                                start=(a == 0), stop=(a == NKC - 1))
                        if c == 0:
                            rope_apply(keyT[0:32, 0, bcols],
                                       keyT[32:64, 0, bcols],
                                       ps[0:32, :], ps[32:64, :], bcols)
                            nc.vector.tensor_copy(out=vk0[0:64, :],
                                                  in_=ps[64:128, :])
                        else:
                            nc.vector.tensor_copy(
                                out=keyT[64:128, c - 1, bcols],
                                in_=ps[0:64, :])
                            nc.vector.tensor_copy(
                                out=keyT[0:64, c, bcols],
                                in_=ps[64:128, :])
                            if c == 1:
                                nc.scalar.copy(out=vk0[64:128, :],
                                               in_=ps[0:64, :])

                    # ---- V assembly via PE transpose (k-major) ----
                    for sc in range(4):
                        tkc = blk * 4 + sc
                        kcols = slice(tok0 + sc * P, tok0 + (sc + 1) * P)
                        lcols = slice(sc * P, (sc + 1) * P)
                        tr = psT.tile([P, 640], bf16, tag="psT", bufs=1,
                                      name="trV")
                        nc.tensor.transpose(tr[:, 0:128], vk0[:, lcols],
                                            identb[:, :])
                        for c in range(1, 4):
                            nc.tensor.transpose(tr[:, c * P:(c + 1) * P],
                                                keyT[:, c, kcols],
                                                identb[:, :])
                        nc.tensor.transpose(tr[:, 512:640],
                                            keyT[:, 4, kcols],
                                            identb[:, :])
                        nc.scalar.copy(out=V[:, tkc, 0:576],
                                       in_=tr[:, 0:576])

                    # ---- fused q projection -> queryT slots (+rope) ----
                    queryT = work.tile([P, 10, SBLK], bf16, tag="queryT",
                                       bufs=1, name="queryT")
                    for c in range(9):
                        ps = psQ.tile([P, SBLK], fp32, tag="psQ", name="psQ")
                        for a in range(NKC):
                            nc.tensor.matmul(
                                ps[:, :], whT[:, a, c * P:(c + 1) * P],
                                hidT[:, a, :],
                                start=(a == 0), stop=(a == NKC - 1))
                        if c == 0:
                            rope_apply(queryT[0:32, 0, :],
                                       queryT[32:64, 0, :],
                                       ps[0:32, :], ps[32:64, :], bcols)
                            nc.scalar.copy(out=queryT[64:128, 0, :],
                                           in_=ps[64:128, :])
                        elif c < 4:
                            nc.scalar.copy(out=queryT[:, c, :], in_=ps[:, :])
                        elif c == 4:
                            nc.scalar.copy(out=queryT[0:64, 4, :],
                                           in_=ps[0:64, :])
                            rope_apply(queryT[0:32, 5, :],
                                       queryT[32:64, 5, :],
                                       ps[64:96, :], ps[96:128, :], bcols)
                        else:
                            nc.vector.tensor_copy(
                                out=queryT[64:128, c, :], in_=ps[0:64, :])
                            nc.vector.tensor_copy(
                                out=queryT[0:64, c + 1, :], in_=ps[64:128, :])

                    if debug and b == 0 and blk == 0:
                        nc.gpsimd.memset(keyT[64:128, 4, :], 0.0)
                        nc.gpsimd.memset(queryT[64:128, 4, :], 0.0)
                        nc.gpsimd.memset(queryT[64:128, 9, :], 0.0)
                        nc.gpsimd.dma_start(out=dbg_v[:, :, :],
                                            in_=V[:, :, :])
                        nc.gpsimd.dma_start(out=dbg_key[:, :, :],
                                            in_=keyT[:, :, :])
                        nc.gpsimd.dma_start(out=dbg_q[:, :, :],
                                            in_=queryT[:, :, :])

                    # ---- attention (2 heads), scores transposed ----
                    attnoutT = work.tile([P, 10, SBLK], bf16, tag="attnoutT",
                                         bufs=1, name="attnoutT")
                    nkt = (blk + 1) * 4
                    for hh in range(HPC if stage >= 2 else 0):
                        avm = [psAV.tile([P, SBLK], fp32, tag="psAV",
                                         name="avm") for _ in range(4)]
                        avt = psT.tile([P, 4, 65], fp32, tag="avt", bufs=1,
                                       name="avt")
                        for kt in range(nkt):
                            ps = psQ.tile([P, SBLK], fp32, tag="psQ",
                                          name="psS")
                            for s_i in range(5):
                                kw = 64 if s_i == 4 else P
                                nc.tensor.matmul(
                                    ps[:, :],
                                    keyT[0:kw, s_i, kt * P:(kt + 1) * P],
                                    queryT[0:kw, hh * 5 + s_i, :],
                                    start=(s_i == 0), stop=(s_i == 4))
                            ept = strm.tile([P, SBLK], bf16, tag="ept",
                                            bufs=3, name="ept")
                            nc.scalar.activation(ept[:, :], ps[:, :], Exp,
                                                 scale=SCALE)
                            if kt // 4 == blk:
                                nc.vector.tensor_mul(
                                    ept[:, :], ept[:, :],
                                    maskT[:, kt % 4, :])
                            if debug and b == 0 and blk == 0 and hh == 0 \
                                    and kt == 0:
                                nc.gpsimd.dma_start(out=dbg_ept[:, :],
                                                    in_=ept[:, :])
                            for qs in range(4):
                                st = (kt == 0)
                                sp = (kt == nkt - 1)
                                nc.tensor.matmul(
                                    avm[qs][:, :],
                                    ept[:, qs * P:(qs + 1) * P],
                                    V[:, kt, 0:512],
                                    start=st, stop=sp, skip_group_check=True)
                                # start=True zero-flags the WHOLE psum bank
                                # (lazily applied on next write), so only the
                                # first group may set it; later qs groups
                                # overwrite via the pending flag it left.
                                nc.tensor.matmul(
                                    avt[:, qs, :],
                                    ept[:, qs * P:(qs + 1) * P],
                                    V[:, kt, 512:577],
                                    start=(st and qs == 0), stop=sp,
                                    skip_group_check=True)
                        if debug and b == 0 and blk == 0 and hh == 0:
                            davt = work.tile([P, 4, 65], fp32, tag="davt",
                                             name="davt")
                            nc.vector.tensor_copy(out=davt[:, :, :],
                                                  in_=avt[:, :, :])
                            nc.gpsimd.dma_start(out=dbg_avt[:, :, :],
                                                in_=davt[:, :, :])
                        ao = work.tile([P, 4, 640], bf16, tag="ao", bufs=2,
                                       name="ao")
                        for qs in range(4):
                            rec = stats.tile([P, 1], fp32, tag="rec",
                                             name="rec")
                            nc.vector.reciprocal(rec[:, :],
                                                 avt[:, qs, 64:65])
                            nc.scalar.activation(ao[:, qs, 0:512],
                                                 avm[qs][:, :], Copy,
                                                 scale=rec[:, :])
                            nc.scalar.activation(ao[:, qs, 512:576],
                                                 avt[:, qs, 0:64], Copy,
                                                 scale=rec[:, :])
                            nc.gpsimd.memset(ao[:, qs, 576:640], 0.0)
                        for qs in range(4):
                            tr = psT.tile([P, 640], bf16, tag="psT", bufs=1,
                                          name="trA")
                            for c in range(5):
                                nc.tensor.transpose(
                                    tr[:, c * P:(c + 1) * P],
                                    ao[:, qs, c * P:(c + 1) * P],
                                    identb[:, :])
                            nc.vector.tensor_copy(
                                out=attnoutT[:, hh * 5:(hh + 1) * 5,
                                             qs * P:(qs + 1) * P],
                                in_=tr[:, :].rearrange("p (a c) -> p a c",
                                                       c=P))

                    # ---- out = attnoutT^T @ w_oT ----
                    for ct in range(HID // SBLK if stage >= 3 else 0):
                        for qs in range(4):
                            ps = psQ.tile([P, SBLK], fp32, tag="psQ",
                                          name="psO")
                            for s_i in range(10):
                                nc.tensor.matmul(
                                    ps[:, :],
                                    attnoutT[:, s_i, qs * P:(qs + 1) * P],
                                    woT[:, s_i, ct * SBLK:(ct + 1) * SBLK],
                                    start=(s_i == 0), stop=(s_i == 9))
                            osb = work.tile([P, SBLK], fp32, tag="osb",
                                            bufs=3, name="osb")
                            if (ct + qs) % 2 == 0:
                                nc.vector.tensor_copy(out=osb[:, :],
                                                      in_=ps[:, :])
                            else:
                                nc.scalar.copy(out=osb[:, :], in_=ps[:, :])
                            nc.gpsimd.dma_start(
                                out=out_d[rows0 + qs * P:
                                          rows0 + (qs + 1) * P,
                                          ct * SBLK:(ct + 1) * SBLK],
                                in_=osb[:, :])

    nc.compile()
    return nc


def make_in_maps(inputs, b_count=B, s_len=S):
    hidden = np.asarray(inputs["hidden_states"],
                        dtype=np.float32).reshape(b_count * s_len, HID)
    cos = np.asarray(inputs["cos"], dtype=np.float32)[0, :s_len]  # [s,64]
    sin = np.asarray(inputs["sin"], dtype=np.float32)[0, :s_len]
    w_qa = np.asarray(inputs["w_qa"], np.float32)
    w_qb = np.asarray(inputs["w_qb"], np.float32)
    w_kv = np.asarray(inputs["w_kv"], np.float32)
    w_o = np.asarray(inputs["w_o"], np.float32)

    hidT = np.ascontiguousarray(hidden.T).astype(BF)            # [HID, R]
    wkvT = np.ascontiguousarray(w_kv.T).astype(BF)              # [HID, 640]
    W_full = w_qb @ w_qa                                        # [H*576, HID]

    # rope table: rows 0:64 cos^T; rows 64:96 sin^T[32:64]; 96:128 sin^T[0:32]
    ropeT = np.ascontiguousarray(np.concatenate(
        [cos.T, sin.T[32:64], sin.T[0:32]], axis=0))            # [128, s]

    r = np.arange(P)[:, None]
    q = np.arange(SBLK)[None, :]
    maskT = np.stack([(r + 128 * j <= q) for j in range(4)],
                     axis=1).astype(BF)                         # [128,4,512]

    in_maps = []
    for c in range(N_CORES):
        W_h = W_full[c * OC:(c + 1) * OC]                       # [1152, HID]
        whT = np.ascontiguousarray(W_h.T).astype(BF)            # [HID, 1152]
        w_o_h = w_o[:, c * OC:(c + 1) * OC]                     # [HID, 1152]
        woT = np.zeros((OCP, HID), np.float32)
        for h2 in range(HPC):
            woT[h2 * 640:h2 * 640 + 576] = \
                w_o_h[:, h2 * 576:(h2 + 1) * 576].T
        in_maps.append({
            "hidT": hidT,
            "whT": whT,
            "wkvT": wkvT,
            "woT": woT.astype(BF),
            "ropeT": ropeT,
            "maskT": maskT,
        })
    return in_maps


_NC_CACHE = {}


def run_on_hw(inputs, trace=False):
    import os

    from concourse.bass_utils import run_bass_kernel_spmd

    if not trace:
        os.environ["BASS_NEVER_TRACE"] = "1"

    key = "full"
    if key not in _NC_CACHE:
        _NC_CACHE[key] = build_nc()
    nc = _NC_CACHE[key]
    in_maps = make_in_maps(inputs)
    res = run_bass_kernel_spmd(nc, in_maps, core_ids=list(range(N_CORES)),
                               trace=trace)
    acc = np.zeros((B * S, HID), dtype=np.float32)
    for r in res.results:
        acc += r["out_part"]
    return acc.reshape(B, S, HID), res


def kernel(**inputs):
    out, _ = run_on_hw(inputs, trace=False)
    return out
